# revision 64
# baseline (speedup 1.0000x reference)
"""MultiHeadAttention kernel for 8 Trainium2 NeuronCores — V2.

Problem: B=2, S=2048, D=512, H=8, per-head full-width projections.

Sharding (V2): batch x query-chunk -> core. Core c owns batch c//4 and
its 512-query chunk (c%4). Every core computes ALL 8 heads for its
queries and accumulates the output projection over heads locally, so
there are NO collectives at all; the host assembles the 8 disjoint
output shards.

Math restructuring (same as V1, verified offline):
  - bk drops out of softmax; bv reduces to a host-side constant row.
  - M  = (Wq[h]/sqrt(D)) @ Wk[h]^T  so scores = q M k^T
    u  = (bq[h]/sqrt(D)) @ Wk[h]^T  per-partition bias on QM^T
    W2 = Wv[h] @ Wo_h               so out += ((attn@v)/den) @ W2
  - No softmax max-subtraction needed (|scores| < ~2.5).

Dtype strategy (numpy-sim predicts HW L2 err to 4 digits; 1.49e-2 vs
2e-2 gate):
  - QM and scores matmuls in fp8e4 with MatmulPerfMode.DoubleRow (2x PE
    rate, measured 128 ns/512-unit vs 239 bf16). fp8e4m3's subnormal
    floor (0.0156) forces scaling: M is stored *M_SCALE and QM^T
    *QTC_SCALE, un-scaled in the bias-add / exp-scale.
  - AV matmul in fp8 DR for NAV8=3 heads (each fp8-AV head adds ~6e-3
    err), bf16 for the rest; output projection always bf16.
  - den reduction tree all-bf16 on DVE (2x rate, err contribution
    ~1e-3 common-mode).

Per-head dataflow (software-pipelined so the PE never waits on DVE):
  QM(h):     QTc[de,q]   = wm_h^T q^T + u    16 bf16 MM -> fp8 via DVE
  scores(h): ps[km,q]    = k QTc             32 fp8 DR-MM -> exp -> PT bf16
  den(h):    DVE chain over PT -> 4 tiny MMs (transpose) -> recipT[q]
  AV(h):     avps[d,q]   = v^T PT            64 bf16 MM (PSUM)
  outproj(h): out[q,do] += (AT_h^T W2_h) * recipT   16 bf16 MM + fused
             scale-accumulate on DVE (scalar_tensor_tensor).
Emission order: ... AV(h) | AT-copies(h), QM(h+1) | outproj(h) |
scores(h+1) | den(h+1), AV(h+1) ... keeps PE saturated.
"""
import os
import sys

sys.path.insert(0, "/opt/trn_rl_repo")
sys.path.insert(0, "/root/.axon_site")

import numpy as np

import concourse.bacc as bacc
import concourse.mybir as mybir
from concourse.tile import TileContext
from concourse import bass_utils

P = 128
B, S, D, H = 2, 2048, 512, 8
NCORES = 8
DT = D // P          # 4 feature tiles
KT = S // P          # 16 key tiles
QC = 512             # queries per core
F32 = mybir.dt.float32
BF16 = mybir.dt.bfloat16
F8 = mybir.dt.float8e4
QTC_SCALE = 32.0     # QTc is stored as fp8 * QTC_SCALE; exp un-scales
M_SCALE = 128.0      # fused M is stored as fp8 * M_SCALE (raw ~6e-4 is
                     # below the fp8 subnormal floor); bias-add rescales
NAV8 = 4             # heads 0..NAV8-1 run the AV matmul in fp8 DoubleRow
                     # (sim: 3 heads -> 1.491e-2, 4 -> 1.613e-2; gate 2e-2
                     # L2 on a fixed, deterministic input)
PARTIAL = {4: 8, 5: 4}   # head -> number of leading kt tiles whose AV
                         # runs in fp8 DR (rest bf16). sim: 1.698e-2

_NC_CACHE = {}


def _build_nc():
    nc = bacc.Bacc("TRN2", target_bir_lowering=False, debug=False,
                   num_devices=NCORES)

    qT = nc.dram_tensor("qT", [D, QC], F8, kind="ExternalInput")
    k8 = nc.dram_tensor("k8", [D, S], F8, kind="ExternalInput")
    v16 = nc.dram_tensor("v16", [S, D], BF16, kind="ExternalInput")
    v8d = nc.dram_tensor("v8d", [S, D], F8, kind="ExternalInput")
    wm = nc.dram_tensor("wm", [H, D, D], F8, kind="ExternalInput")
    w2 = nc.dram_tensor("w2", [H, D, D], BF16, kind="ExternalInput")
    # uv pre-transposed on host to [P, H, DT] so the DMA is contiguous
    # (the natural [H, D] layout scatters into 4-byte runs — ~25us DMA)
    uv = nc.dram_tensor("uv", [P, H, DT], F32, kind="ExternalInput")
    oinv = nc.dram_tensor("oinv", [P, 2], BF16, kind="ExternalInput")
    out = nc.dram_tensor("out", [QC, D], F32, kind="ExternalOutput")

    Add = mybir.AluOpType.add
    Mult = mybir.AluOpType.mult
    DR = mybir.MatmulPerfMode.DoubleRow

    with TileContext(nc) as tc:
        with (
            tc.tile_pool(name="consts", bufs=1) as consts,
            tc.tile_pool(name="qtc", bufs=2) as qtcp,
            tc.tile_pool(name="pt", bufs=2) as ptp,
            tc.tile_pool(name="pt8", bufs=2) as ptp8,
            tc.tile_pool(name="at", bufs=2) as atp,
            tc.tile_pool(name="small", bufs=3) as small,
            tc.tile_pool(name="dtree", bufs=1) as dtree,
            tc.tile_pool(name="rot", bufs=4, space="PSUM") as rot,
            tc.tile_pool(name="avp", bufs=1, space="PSUM") as avp,
        ):
            # ---- constant loads, startup-critical first. dma_start costs
            # ~650ns of ISSUE time on the issuing engine's queue, so the
            # early loads are spread across idle engine queues.
            u_sb = consts.tile([P, H, DT], F32, name="u_sb")
            nc.sync.dma_start(u_sb[:], uv[:])
            # HAM pre-warm: the PE clock-gate defaults to 1.2 GHz and takes
            # ~3.4us of sustained activity to release. Burn dummy matmuls on
            # a memset tile while the first input DMAs are in flight so the
            # real stream starts at 2.4 GHz.
            warm_sb = consts.tile([P, P], BF16, name="warm_sb")
            nc.vector.memset(warm_sb[:], 0)
            warm_ps = rot.tile([P, QC], F32, tag="ps")
            for _w in range(24):
                nc.tensor.matmul(warm_ps[:, 0:P], lhsT=warm_sb[:],
                                 rhs=warm_sb[:], start=True, stop=True)
            oinv_sb = consts.tile([P, 2], BF16, name="oinv_sb")
            nc.scalar.dma_start(oinv_sb[:], oinv[:])

            wm_sb = consts.tile([P, H, DT, D], F8, name="wm_sb")
            w2_sb = consts.tile([P, H, DT, D], BF16, name="w2_sb")
            q_sb = consts.tile([P, DT, QC], F8, name="q_sb")
            k_sb = consts.tile([P, DT, S], F8, name="k_sb")
            v_sb = consts.tile([P, KT, D], BF16, name="v_sb")
            v8_sb = consts.tile([P, KT, D], F8, name="v8_sb")
            outacc = consts.tile([P, DT, D], F32, name="outacc")

            def load_head_w(dst, src, h):
                nc.sync.dma_start(
                    dst[:, h], src[h].rearrange("(t p) e -> p t e", p=P))

            # first-head weights + q issued in parallel from idle queues so
            # the first QM matmul starts as early as possible
            wm0ap = wm[0].rearrange("(t p) e -> p t e", p=P)
            qap = qT[:].rearrange("(t p) q -> p t q", p=P)
            nc.gpsimd.dma_start(wm_sb[:, 0], wm0ap)
            nc.sync.dma_start(q_sb[:, 0:2], qap[:, 0:2])
            nc.gpsimd.dma_start(q_sb[:, 2:4], qap[:, 2:4])
            # k in quarters so scores can start before all of k lands
            kap = k8[:].rearrange("(t p) s -> p t s", p=P)
            nc.scalar.dma_start(k_sb[:, :, 0:S // 4], kap[:, :, 0:S // 4])
            for qtr in range(1, 4):
                sl = slice(qtr * (S // 4), (qtr + 1) * (S // 4))
                nc.sync.dma_start(k_sb[:, :, sl], kap[:, :, sl])
            v8ap = v8d[:].rearrange("(t p) d -> p t d", p=P)
            for half in range(2):
                sl = slice(half * (KT // 2), (half + 1) * (KT // 2))
                nc.gpsimd.dma_start(v8_sb[:, sl], v8ap[:, sl])
            load_head_w(w2_sb, w2, 0)
            vap = v16[:].rearrange("(t p) d -> p t d", p=P)

            def emit_deferred_loads(h):
                # spread the remaining issue cost across the head loop;
                # each load lands several heads before it is consumed
                if h <= 3:
                    # quarter h lands well before the first bf16-AV head
                    # (h=NAV8) consumes it
                    sl = slice(h * (KT // 4), (h + 1) * (KT // 4))
                    nc.gpsimd.dma_start(v_sb[:, sl], vap[:, sl])
                if h + 1 < H:
                    nc.gpsimd.dma_start(
                        wm_sb[:, h + 1],
                        wm[h + 1].rearrange("(t p) e -> p t e", p=P))
                    load_head_w(w2_sb, w2, h + 1)

            # ---- per-head emission helpers
            def emit_qm(h):
                QTc = qtcp.tile([P, DT, QC], F8, tag="QT")
                for et in range(DT):
                    ps = rot.tile([P, QC], F32, tag="ps")
                    for p2 in range(2):
                        nc.tensor.matmul(
                            ps[:],
                            lhsT=wm_sb[:, h, 2 * p2:2 * p2 + 2,
                                       et * P:(et + 1) * P],
                            rhs=q_sb[:, 2 * p2:2 * p2 + 2, :],
                            start=(p2 == 0), stop=(p2 == 1),
                            perf_mode=DR,
                        )
                    # ps is M_SCALE times the true QM; QTc = (ps + u*M_SCALE)
                    # * (QTC_SCALE/M_SCALE) in fp8; exp un-scales QTC_SCALE.
                    # Host passes uv pre-multiplied by M_SCALE.
                    nc.vector.tensor_scalar(
                        QTc[:, et, :], ps[:], u_sb[:, h, et:et + 1],
                        float(QTC_SCALE / M_SCALE), Add, Mult)
                return QTc

            def emit_scores(QTc, h):
                if h in PARTIAL:
                    sp = PARTIAL[h]
                    # shared max-size tiles for all partial heads
                    PT8m = ptp8.tile([P, 8, QC], F8, tag="PT8m")
                    PT16m = ptp.tile([P, 12, QC], BF16, tag="PT16m")
                    for kt in range(KT):
                        ps = rot.tile([P, QC], F32, tag="ps")
                        for p2 in range(2):
                            nc.tensor.matmul(
                                ps[:],
                                lhsT=k_sb[:, 2 * p2:2 * p2 + 2,
                                          kt * P:(kt + 1) * P],
                                rhs=QTc[:, 2 * p2:2 * p2 + 2, :],
                                start=(p2 == 0), stop=(p2 == 1),
                                perf_mode=DR,
                            )
                        dst = (PT8m[:, kt, :] if kt < sp
                               else PT16m[:, kt - sp, :])
                        nc.scalar.activation(
                            dst, ps[:],
                            mybir.ActivationFunctionType.Exp,
                            scale=1.0 / QTC_SCALE)
                    return (PT8m, PT16m, sp)
                if h < NAV8:
                    PT = ptp8.tile([P, KT, QC], F8, tag="PT8")
                else:
                    PT = ptp.tile([P, KT, QC], BF16, tag="PT")
                for kt in range(KT):
                    ps = rot.tile([P, QC], F32, tag="ps")
                    for p2 in range(2):
                        nc.tensor.matmul(
                            ps[:],
                            lhsT=k_sb[:, 2 * p2:2 * p2 + 2,
                                      kt * P:(kt + 1) * P],
                            rhs=QTc[:, 2 * p2:2 * p2 + 2, :],
                            start=(p2 == 0), stop=(p2 == 1),
                            perf_mode=DR,
                        )
                    nc.scalar.activation(
                        PT[:, kt, :], ps[:],
                        mybir.ActivationFunctionType.Exp,
                        scale=1.0 / QTC_SCALE)
                return PT

            def emit_den(PT):
                # balanced reduction tree, all-16-bit DVE ops (2x rate);
                # the lost den precision (~1e-3, common-mode) is negligible.
                # For the half head PT is (PT8h, PT16h); level-1 pairs never
                # cross the dtype boundary.
                if isinstance(PT, tuple):
                    PT8m, PT16m, sp = PT

                    def pt(i):
                        return (PT8m[:, i, :] if i < sp
                                else PT16m[:, i - sp, :])
                else:
                    def pt(i):
                        return PT[:, i, :]
                dt8 = dtree.tile([P, 8, QC], BF16, tag="dtree")
                for i in range(8):
                    nc.vector.tensor_add(dt8[:, i, :], pt(2 * i),
                                         pt(2 * i + 1))
                for i in range(4):
                    nc.vector.tensor_add(dt8[:, 2 * i, :], dt8[:, 2 * i, :],
                                         dt8[:, 2 * i + 1, :])
                nc.vector.tensor_add(dt8[:, 0, :], dt8[:, 0, :], dt8[:, 2, :])
                nc.vector.tensor_add(dt8[:, 4, :], dt8[:, 4, :], dt8[:, 6, :])
                denB = small.tile([P, QC], BF16, tag="denB")
                nc.vector.tensor_add(denB[:], dt8[:, 0, :], dt8[:, 4, :])
                return denB

            def emit_recip(denB):
                # transpose den via 4 tiny bf16 MMs, then reciprocal
                denT = rot.tile([P, QC], F32, tag="ps")
                for t in range(4):
                    nc.tensor.matmul(
                        denT[:, 2 * t:2 * t + 2],
                        lhsT=denB[:, t * P:(t + 1) * P],
                        rhs=oinv_sb[:],
                        start=True, stop=True,
                    )
                recipT = small.tile([P, 8], F32, tag="recipT")
                nc.vector.reciprocal(recipT[:], denT[:, 0:8])
                return recipT

            def emit_av_first(PT, h, upto):
                av = avp.tile([P, DT, QC], F32, tag="av")
                if h < NAV8:
                    for j in range(upto // 2):
                        for et in range(DT):
                            nc.tensor.matmul(
                                av[:, et, :],
                                lhsT=v8_sb[:, 2 * j:2 * j + 2,
                                           et * P:(et + 1) * P],
                                rhs=PT[:, 2 * j:2 * j + 2, :],
                                start=(j == 0), stop=False,
                                perf_mode=DR,
                            )
                else:
                    for kt in range(upto):
                        for et in range(DT):
                            nc.tensor.matmul(
                                av[:, et, :],
                                lhsT=v_sb[:, kt, et * P:(et + 1) * P],
                                rhs=PT[:, kt, :],
                                start=(kt == 0), stop=False,
                            )
                return av

            def emit_av_rest(av, PT, h, frm):
                if h < NAV8:
                    for j in range(frm // 2, KT // 2):
                        for et in range(DT):
                            nc.tensor.matmul(
                                av[:, et, :],
                                lhsT=v8_sb[:, 2 * j:2 * j + 2,
                                           et * P:(et + 1) * P],
                                rhs=PT[:, 2 * j:2 * j + 2, :],
                                start=False, stop=(j == KT // 2 - 1),
                                perf_mode=DR,
                            )
                else:
                    for kt in range(frm, KT):
                        for et in range(DT):
                            nc.tensor.matmul(
                                av[:, et, :],
                                lhsT=v_sb[:, kt, et * P:(et + 1) * P],
                                rhs=PT[:, kt, :],
                                start=False, stop=(kt == KT - 1),
                            )

            def emit_at_copies(av):
                # on ACT (scalar) engine: DVE is the busier one
                AT = atp.tile([P, DT, QC], BF16, tag="AT")
                for et in range(DT):
                    nc.scalar.activation(
                        AT[:, et, :], av[:, et, :],
                        mybir.ActivationFunctionType.Copy)
                return AT

            def emit_outproj(h, AT, recipT):
                for t in range(4):
                    ps = rot.tile([P, QC], F32, tag="ps")
                    for et in range(DT):
                        nc.tensor.matmul(
                            ps[:],
                            lhsT=AT[:, et, t * P:(t + 1) * P],
                            rhs=w2_sb[:, h, et, :],
                            start=(et == 0), stop=(et == DT - 1),
                        )
                    if h == 0:
                        nc.vector.tensor_scalar_mul(
                            outacc[:, t, :], ps[:], recipT[:, 2 * t:2 * t + 1])
                    else:
                        nc.vector.scalar_tensor_tensor(
                            outacc[:, t, :], ps[:],
                            recipT[:, 2 * t:2 * t + 1], outacc[:, t, :],
                            Mult, Add)
                    if h == H - 1:
                        # stream each finished row-block out immediately
                        nc.sync.dma_start(out[t * P:(t + 1) * P, :],
                                          outacc[:, t, :])

            # ---- software-pipelined head loop
            QTc = emit_qm(0)
            PT = emit_scores(QTc, 0)
            denAcc = emit_den(PT)
            state = (PT, denAcc)
            for h in range(H):
                PT, denAcc = state
                emit_deferred_loads(h)
                # AV split so the tiny den-transpose MMs land mid-AV,
                # late enough that the den reduction tree has finished
                if h in PARTIAL:
                    PT8m, PT16m, sp = PT
                    av = avp.tile([P, DT, QC], F32, tag="av")
                    for j in range(sp // 2):    # kt 0..sp-1 in fp8 DR
                        for et in range(DT):
                            nc.tensor.matmul(
                                av[:, et, :],
                                lhsT=v8_sb[:, 2 * j:2 * j + 2,
                                           et * P:(et + 1) * P],
                                rhs=PT8m[:, 2 * j:2 * j + 2, :],
                                start=(j == 0), stop=False,
                                perf_mode=DR,
                            )
                    # den tree (exp-paced) needs ~3.5us of AV before the
                    # transpose MMs; pad with bf16 kts when sp is small
                    pad = sp if sp >= 8 else sp + 2
                    for kt in range(sp, pad):
                        for et in range(DT):
                            nc.tensor.matmul(
                                av[:, et, :],
                                lhsT=v_sb[:, kt, et * P:(et + 1) * P],
                                rhs=PT16m[:, kt - sp, :],
                                start=False, stop=False,
                            )
                    recipT = emit_recip(denAcc)
                    for kt in range(pad, KT):   # rest in bf16
                        for et in range(DT):
                            nc.tensor.matmul(
                                av[:, et, :],
                                lhsT=v_sb[:, kt, et * P:(et + 1) * P],
                                rhs=PT16m[:, kt - sp, :],
                                start=False, stop=(kt == KT - 1),
                            )
                else:
                    cut = 10 if h < NAV8 else 6
                    av = emit_av_first(PT, h, upto=cut)
                    recipT = emit_recip(denAcc)
                    emit_av_rest(av, PT, h, frm=cut)
                AT = emit_at_copies(av)
                if h + 1 < H:
                    QTc = emit_qm(h + 1)
                emit_outproj(h, AT, recipT)
                if h + 1 < H:
                    PT = emit_scores(QTc, h + 1)
                    denAcc = emit_den(PT)
                    state = (PT, denAcc)

    nc.compile()
    return nc


def kernel(q, k, v, Wq, Wk, Wv, bq, bk, bv, Wo, bo):
    import ml_dtypes

    if "nc" not in _NC_CACHE:
        _NC_CACHE["nc"] = _build_nc()
    nc = _NC_CACHE["nc"]

    q = np.asarray(q, dtype=np.float32)
    k = np.asarray(k, dtype=np.float32)
    v = np.asarray(v, dtype=np.float32)
    Wq = np.asarray(Wq, dtype=np.float32)
    Wk = np.asarray(Wk, dtype=np.float32)
    Wv = np.asarray(Wv, dtype=np.float32)
    bq = np.asarray(bq, dtype=np.float32)
    bv = np.asarray(bv, dtype=np.float32)
    Wo = np.asarray(Wo, dtype=np.float32)
    bo = np.asarray(bo, dtype=np.float32)

    bf16 = ml_dtypes.bfloat16
    f8 = ml_dtypes.float8_e4m3

    def cbf(x):
        return np.ascontiguousarray(x.astype(bf16))

    def cf8(x):
        return np.ascontiguousarray(x.astype(f8))

    scale = np.float32(1.0 / np.sqrt(D))
    m_s = np.float32(M_SCALE)
    wm_np = cf8(np.stack([(Wq[h] * (scale * m_s)) @ Wk[h].T
                          for h in range(H)]))
    w2_np = cbf(np.stack([Wv[h] @ Wo[h * D:(h + 1) * D, :]
                          for h in range(H)]))
    uv_raw = np.stack([(bq[h] * (scale * m_s)) @ Wk[h].T
                       for h in range(H)])          # [H, D]
    uv_np = np.ascontiguousarray(                   # -> [P, H, DT]
        uv_raw.reshape(H, DT, P).transpose(2, 0, 1).astype(np.float32))
    oinv_np = np.ones((P, 2), dtype=bf16)

    k8 = [cf8(k[b].T) for b in range(B)]
    v16 = [cbf(v[b]) for b in range(B)]
    v8 = [cf8(v[b]) for b in range(B)]

    in_maps = []
    for c in range(NCORES):
        b, qi = c // 4, c % 4
        in_maps.append({
            "qT": cf8(q[b, qi * QC:(qi + 1) * QC, :].T),
            "k8": k8[b],
            "v16": v16[b],
            "v8d": v8[b],
            "wm": wm_np,
            "w2": w2_np,
            "uv": uv_np,
            "oinv": oinv_np,
        })

    trace = bool(int(os.environ.get("KERNEL_TRACE", "0")))
    if trace:
        try:
            import trace_hook
            trace_hook.install()
        except Exception:
            pass
    res = bass_utils.run_bass_kernel_spmd(
        nc, in_maps, core_ids=list(range(NCORES)), trace=trace
    )
    _NC_CACHE["last_result"] = res

    out = np.empty((B, S, D), dtype=np.float32)
    for c in range(NCORES):
        b, qi = c // 4, c % 4
        out[b, qi * QC:(qi + 1) * QC, :] = np.array(res.results[c]["out"])
    c_const = sum(bv[h] @ Wo[h * D:(h + 1) * D, :] for h in range(H)) + bo
    out += c_const[None, None, :].astype(np.float32)
    return out
